# revision 32
# baseline (speedup 1.0000x reference)
"""Trainium2 Bass kernel for nn_Attention_65609920414302 (sparse multi-branch attention).

Sharding: 64 total heads (4 branches x 16 sub-heads) split as 8 heads per core
(core c = branch c//2, base-head half c%2). Each core computes Q/K/V projections
for its heads, RoPE, causal thresholded-softplus attention, and a partial W_O
matmul; the host sums the 8 partial outputs.

Math rescaling used on device (S = pi/sqrt(3)):
  reference w_sig = w*sigmoid(S*w) with w = softplus(scores*m), thresholded at sink.
  device   W = silu(S*w) = S*w_sig, thresholded at S*sink,
  probs    = W / (sum_s W + S*(sink+1e-6)),  sink term = S*sink / (...).
The S factors cancel exactly. softplus is composed as ln(1 + exp(x)) since this
toolchain has no softplus ACT table.

v3 pipeline (vs the phase-serial baseline):
  - per-group software pipeline K(g)->m(g)->Q(g)->scores(g)/exp(g) so the ACT
    engine starts early and runs a dense exp/ln/silu stream (ACT is the
    bottleneck engine: 3 passes x 36864 cols at ~1 col/cycle @1.2GHz).
  - 9 batched input DMAs (two constant blobs; xt/wk split in halves) instead
    of 39: the SP queue issues one DMA per ~0.7us, so DMA count is front
    latency.
  - the per-key scale m = 1/(8*sqrt(key_self)) is premultiplied into krope
    (tiny PE outer-product broadcast + DVE mult) so exp needs no per-partition
    scale, allowing score blocks packed into [128,1024] PSUM tiles (5/head).
  - m = exp(0.5*ln(r/64)) on the same ln/exp ACT table (no sqrt-table load).
  - causal masking of each diagonal block is an additive -60000 upper-triangle
    matmul into the score PSUM accumulation (no Pool/DVE masking pass at all;
    exp then produces exact zeros).
  - Q/K biases folded into the PSUM->SBUF copy (per-partition tensor_scalar).
  - W_O runs in 3 rounds of 8 [128,1024] units: ct0+ct1 (PSUM pair) -> y_acc,
    ct2 -> add, ct3 -> final add -> DMA.
  - HAM (power management) halves every engine's clock when the PE idles, so
    dummy warm matmul streams fill the known PE gaps (tail chains etc.).
"""

import math
import os
import numpy as np

D_MODEL = 1024
N_HEAD = 16
N_BR = 4
DH = 64
H_TOT = 64
T = 1024
S = math.pi / math.sqrt(3.0)
N_CORES = 8
HPC = 8          # heads per core
KT = 8           # C // 128 contraction tiles
W_COLS = 4608    # sum of causal-trapezoid block widths

# packed block order inside wbuf: pairs that sum to 1024 columns share a PSUM
# tile: (0), (1,7), (2,6), (3,5), (4)
BLK_ORDER = [0, 1, 7, 2, 6, 3, 5, 4]
BLK_LEN = {i: T - 128 * i for i in range(8)}
OFF = {}
_cur = 0
for _b in BLK_ORDER:
    OFF[_b] = _cur
    _cur += BLK_LEN[_b]
assert _cur == W_COLS
TILE_GROUPS = [(0,), (1, 7), (2, 6), (3, 5), (4,)]

# f16 constant blob column layout
CB_ONES = 0        # [1, 512] row 0
CB_PSW = 512       # [128, 128]
CB_SEL2 = 640      # [128, 8] (4 groups x 2)
CB_BCH = 648       # [2, 128] rows 0-1
CB_BV = 776        # [1, 512] row 0
CB_COS = 1288      # [128, 1024]
CB_SIN = 2312      # [128, 1024]
CB_IDF = 3336      # [128, 128] identity
CB_MSK = 3464      # [128, 128] -60000 upper triangle (key > query)
CB16_COLS = 3592

_NC_CACHE = [None]
LAST_RESULT = [None]  # stash for test harness (exec_time_ns etc.)


def _build_nc():
    import concourse.bass as bass
    from concourse import bacc
    import concourse.mybir as mybir
    import concourse.tile as tile
    from concourse.tile import add_dep_helper

    F32 = mybir.dt.float32
    F32R = mybir.dt.float32r
    F16 = mybir.dt.float16
    AF = mybir.ActivationFunctionType
    ALU = mybir.AluOpType

    nc = bacc.Bacc(None, target_bir_lowering=False, debug=False)

    # ---- DRAM parameters (per-core data; same program on all cores) ----
    XT = nc.declare_dram_parameter("XT", [D_MODEL, T], F16, isOutput=False)
    WQ = nc.declare_dram_parameter("WQ", [D_MODEL, 512], F16, isOutput=False)
    WK = nc.declare_dram_parameter("WK", [D_MODEL, 512], F16, isOutput=False)
    WV = nc.declare_dram_parameter("WV", [D_MODEL, 512], F16, isOutput=False)
    CB16 = nc.declare_dram_parameter("CB16", [128, CB16_COLS], F16, isOutput=False)
    CB32 = nc.declare_dram_parameter("CB32", [128, 32], F32, isOutput=False)
    WO = nc.declare_dram_parameter("WO", [512, D_MODEL], F16, isOutput=False)
    YT = nc.declare_dram_parameter("YT", [D_MODEL, T], F32, isOutput=True)
    dbg = bool(os.environ.get("KDEBUG"))
    if dbg:
        DKR = nc.declare_dram_parameter("DKR", [128, 4, T], F16, isOutput=True)
        DQR = nc.declare_dram_parameter("DQR", [128, 4, T], F16, isOutput=True)
        DWB = nc.declare_dram_parameter("DWB", [128, 2, W_COLS], F16, isOutput=True)
        DCTX = nc.declare_dram_parameter("DCTX", [128, 4, T], F16, isOutput=True)

    with tile.TileContext(nc) as tc:
        pc = tc.alloc_tile_pool(name="const", bufs=1)
        pk = tc.alloc_tile_pool(name="keep", bufs=1)
        tr = tc.alloc_tile_pool(name="trans", bufs=2)
        pw = tc.alloc_tile_pool(name="wbuf", bufs=3)
        pa = tc.alloc_tile_pool(name="psall", bufs=1, space="PSUM")

        # ---- weights + constants (9 DMAs; xt/wk halves first) ----
        xt = pk.tile([128, KT, T], F16)
        wq = pk.tile([128, KT, 4, 128], F16)
        wk = pk.tile([128, KT, 4, 128], F16)
        wv = pk.tile([128, KT, 512], F16)
        cb16 = pc.tile([128, CB16_COLS], F16)
        cb32 = pc.tile([128, 32], F32)
        wo = pk.tile([128, 4, 8, 128], F16)

        xt_src = XT.ap().rearrange("(kt p) t -> p kt t", p=128)
        wk_src = WK.ap().rearrange("(kt p) (mt m) -> p kt mt m", p=128, m=128)
        nc.sync.dma_start(out=cb16, in_=CB16.ap())
        nc.sync.dma_start(out=xt[:, 0:4, :], in_=xt_src[:, 0:4, :])
        nc.sync.dma_start(out=wk[:, 0:4, :, :], in_=wk_src[:, 0:4, :, :])
        nc.sync.dma_start(out=xt[:, 4:8, :], in_=xt_src[:, 4:8, :])
        nc.sync.dma_start(out=wk[:, 4:8, :, :], in_=wk_src[:, 4:8, :, :])
        nc.sync.dma_start(out=cb32, in_=CB32.ap())
        nc.sync.dma_start(
            out=wq, in_=WQ.ap().rearrange("(kt p) (mt m) -> p kt mt m", p=128, m=128)
        )
        nc.sync.dma_start(out=wv, in_=WV.ap().rearrange("(kt p) v -> p kt v", p=128))
        nc.sync.dma_start(
            out=wo, in_=WO.ap().rearrange("(ct p) (mt m) -> p ct mt m", p=128, m=128)
        )

        # constant views into the blobs
        ones_r = cb16[0:1, CB_ONES:CB_ONES + 512]
        psw_sb = cb16[:, CB_PSW:CB_PSW + 128]
        bch_sb = cb16[0:2, CB_BCH:CB_BCH + 128]
        bv = cb16[0:1, CB_BV:CB_BV + 512]
        cos_sb = cb16[:, CB_COS:CB_COS + T]
        sin_sb = cb16[:, CB_SIN:CB_SIN + T]
        idf_sb = cb16[:, CB_IDF:CB_IDF + 128]
        msk_sb = cb16[:, CB_MSK:CB_MSK + 128]
        thr_sb = cb32[:, 0:8]
        tb_sb = cb32[0:1, 8:16]
        vns_sb = cb32[0:64, 16:24]
        bqt_sb = cb32[:, 24:28]
        bkt_sb = cb32[:, 28:32]

        def sel2_sb(g):
            return cb16[:, CB_SEL2 + 2 * g:CB_SEL2 + 2 * g + 2]

        qrope = pk.tile([128, 4, T], F16)
        krope = pk.tile([128, 4, T], F16)
        vstore = pk.tile([128, 8, HPC, 65], F16)
        ctx = pk.tile([128, 4, T], F16)
        y_acc = pk.tile([128, 8, T], F16)
        nc.vector.memset(vstore[:, :, :, 64:65], 1.0)

        # warm-up / clock-hold matmuls: HAM halves every engine's clock when
        # the PE idles, so dummy streams fill known PE gaps. The warm source
        # is memset (not DMA) so warm-ups start before any DMA lands.
        wsrc = pc.tile([1, 512], F16)
        nc.vector.memset(wsrc, 1.0)

        def warm_pe(n):
            wu = pa.tile([1, 512], F32, tag="pvproj", bufs=2)
            for _ in range(n):
                nc.tensor.matmul(wu, wsrc[0:1, 0:1], wsrc, start=True,
                                 stop=True)

        warm_pe(6)

        # ---------- emission helpers ----------

        def proj_chain(w_t, g):
            """X @ W slice for group g -> [128, T] PSUM tile."""
            ps = pa.tile([128, T], F32, tag="pvproj", bufs=2)
            for th in range(2):
                sl = slice(512 * th, 512 * (th + 1))
                for kt in range(KT):
                    nc.tensor.matmul(
                        ps[:, sl], w_t[:, kt, g, :], xt[:, kt, sl],
                        start=(kt == 0), stop=(kt == KT - 1),
                    )
            return ps

        def bias_copy(ps, bias_t, g):
            """PSUM -> f16 SBUF copy with the per-partition bias folded in
            (frees the projection's PSUM slot early)."""
            qsb = tr.tile([128, T], F16, tag="qsb")
            nc.vector.tensor_scalar_add(qsb, ps, bias_t[:, g:g + 1])
            return qsb

        def rope_from(qsb, g, out_t):
            """swap matmul, cos/sin muls, combine -> out_t[:, g, :]."""
            sw = pa.tile([128, T], F32, tag="pvproj", bufs=2)
            for th in range(2):
                sl = slice(512 * th, 512 * (th + 1))
                nc.tensor.matmul(sw[:, sl], psw_sb, qsb[:, sl], start=True,
                                 stop=True)
            t1 = tr.tile([128, T], F16, tag="t1", bufs=1)
            nc.gpsimd.tensor_tensor(t1, qsb, cos_sb, op=ALU.mult)
            t2 = tr.tile([128, T], F16, tag="t2")
            nc.vector.tensor_tensor(t2, sw, sin_sb, op=ALU.mult)
            nc.gpsimd.tensor_tensor(out_t[:, g, :], t1, t2, op=ALU.add)

        def finish_k_phase(g, ksb):
            """key_self -> m chain + K rope + krope scaling."""
            rope_from(ksb, g, krope)
            # key_self from the pre-RoPE projection (rotation-invariant)
            k2 = tr.tile([128, T], F16, tag="k2", bufs=1)
            nc.gpsimd.tensor_tensor(k2, ksb, ksb, op=ALU.mult)
            ks = pa.tile([2, T], F32, tag="pvproj", bufs=2)
            for th in range(2):
                sl = slice(512 * th, 512 * (th + 1))
                nc.tensor.matmul(ks[:, sl], sel2_sb(g), k2[:, sl],
                                 start=True, stop=True)
            mx = tr.tile([2, T], F32, tag="mx", bufs=1)
            nc.vector.tensor_scalar_max(mx, ks, 1e-6)
            nc.vector.reciprocal_approx_fast(mx, mx)
            # m = sqrt(r/64) = exp(0.5*ln(r/64)), on the shared ln/exp table
            nc.scalar.activation(mx, mx, AF.Ln, scale=1.0 / 64.0)
            m2 = tr.tile([2, T], F16, tag="m2", bufs=1)
            nc.scalar.activation(m2, mx, AF.Exp, scale=0.5)
            # broadcast m to both 64-row halves via a tiny PE outer product,
            # then fold into krope with a single full-width multiply
            mb = pa.tile([128, T], F32, tag="pvproj", bufs=2)
            for th in range(2):
                sl = slice(512 * th, 512 * (th + 1))
                nc.tensor.matmul(mb[:, sl], bch_sb, m2[:, sl], start=True,
                                 stop=True)
            nc.vector.tensor_tensor(krope[:, g, :], krope[:, g, :], mb,
                                    op=ALU.mult)



        def score_tile_units(g):
            """Per-PSUM-tile closures for wave g's scores + causal mask + exp."""
            wbuf = wbuf_of[g]
            units = []
            for ti, blocks in enumerate(TILE_GROUPS):
                for u in range(2):
                    units.append((blocks, u, 64 * u))

            def emit_unit(idx):
                blocks, u, r0 = units[idx]
                ncols = sum(BLK_LEN[b] for b in blocks)
                woff = OFF[blocks[0]]
                ps_s = pa.tile([128, T], F32, tag="scores", bufs=2)
                p0 = 0
                for b in blocks:
                    L = BLK_LEN[b]
                    t0 = 128 * b
                    # diagonal 128-col chunk first (open accumulation group),
                    # immediately closed by the additive causal mask matmul
                    # (-60000 where key > query; exp then yields exact zeros),
                    # then the remaining 512-boundary splits as closed groups
                    nc.tensor.matmul(
                        ps_s[:, p0:p0 + 128],
                        krope[r0:r0 + 64, g, t0:t0 + 128],
                        qrope[r0:r0 + 64, g, t0:t0 + 128],
                        start=True, stop=False,
                    )
                    nc.tensor.matmul(ps_s[:, p0:p0 + 128], idf_sb, msk_sb,
                                     start=False, stop=True)
                    c = p0 + 128
                    while c < p0 + L:
                        nxt = min(((c // 512) + 1) * 512, p0 + L)
                        nc.tensor.matmul(
                            ps_s[:, c:nxt],
                            krope[r0:r0 + 64, g, t0:t0 + 128],
                            qrope[r0:r0 + 64, g, t0 + (c - p0):t0 + (nxt - p0)],
                            start=True, stop=True,
                        )
                        c = nxt
                    p0 += L
                e = nc.scalar.activation(
                    wbuf[:, u, woff:woff + ncols], ps_s[:, 0:ncols], AF.Exp
                )
                exp_insts_of[g].append(e)
                for si in silu_of.get(g - 1, []):
                    add_dep_helper(e.ins, si.ins, sync=False,
                                   reason="act table phase order")

            return [lambda i=i: emit_unit(i) for i in range(len(units))]

        def emit_ln_silu(g):
            wbuf = wbuf_of[g]
            ln = nc.scalar.activation(wbuf[:, :, :], wbuf[:, :, :], AF.Ln,
                                      bias=1.0)
            for e in exp_insts_of[g]:
                add_dep_helper(ln.ins, e.ins, sync=False,
                               reason="act table phase order")
            silu_of[g] = []
            for u in range(2):
                si = nc.scalar.activation(wbuf[:, u, :], wbuf[:, u, :], AF.Silu,
                                          scale=S)
                add_dep_helper(si.ins, ln.ins, sync=False,
                               reason="act table phase order")
                silu_of[g].append(si)

        def emit_head_tail(g, u):
            """threshold + PV + normalize for head h = 2g+u -> ctx."""
            h = 2 * g + u
            r0 = 64 * u
            wbuf = wbuf_of[g]
            nc.vector.scalar_tensor_tensor(
                out=wbuf[:, u, :], in0=wbuf[:, u, :],
                scalar=thr_sb[:, h:h + 1], in1=wbuf[:, u, :],
                op0=ALU.is_ge, op1=ALU.mult,
            )
            ps_pv = pa.tile([65, T], F32, tag="pvproj", bufs=2)
            for i in range(8):
                t0 = 128 * i
                o = OFF[i]
                if t0 < 512:
                    chunks = [(t0, 512, 3), (512, T, 7)]
                else:
                    chunks = [(t0, T, 7)]
                for (a, b, last_i) in chunks:
                    nc.tensor.matmul(
                        ps_pv[:, a:b],
                        vstore[:, i, h, :],
                        wbuf[:, u, o + (a - t0):o + (b - t0)],
                        start=(i == 0), stop=(i == last_i),
                    )
            tp = tr.tile([1, T], F32, tag="tp", bufs=1)
            nc.vector.tensor_scalar_add(tp, ps_pv[64:65, :],
                                        tb_sb[0:1, h:h + 1])
            nc.vector.reciprocal_approx_fast(tp, tp)
            gb = tr.tile([64, T], F32, tag="gb", bufs=1)
            nc.gpsimd.partition_broadcast(gb, tp, channels=64)
            nc.vector.scalar_tensor_tensor(
                out=ctx[r0:r0 + 64, g, :], in0=ps_pv[0:64, :],
                scalar=vns_sb[:, h:h + 1], in1=gb,
                op0=ALU.add, op1=ALU.mult,
            )

        def emit_v_unit(tt):
            psv = pa.tile([128, 512], F32, tag="pvproj", bufs=2)
            for kt in range(KT):
                nc.tensor.matmul(
                    psv, xt[:, kt, 128 * tt:128 * (tt + 1)],
                    wv[:, kt, :], start=(kt == 0), stop=False,
                )
            nc.tensor.matmul(psv, ones_r[0:1, 0:128], bv, start=False,
                             stop=True)
            nc.vector.tensor_copy(
                vstore[:, tt, :, 0:64],
                psv.rearrange("p (h d) -> p h d", d=64),
            )

        def emit_wo_round_a():
            for mt in range(8):
                po = pa.tile([128, T], F32, tag="pvproj", bufs=2)
                for th in range(2):
                    sl = slice(512 * th, 512 * (th + 1))
                    for ci, ct in enumerate((0, 1)):
                        nc.tensor.matmul(
                            po[:, sl], wo[:, ct, mt, :], ctx[:, ct, sl],
                            start=(ci == 0), stop=(ci == 1),
                        )
                nc.vector.tensor_copy(y_acc[:, mt, :], po)

        def emit_wo_round_b():
            for mt in range(8):
                po = pa.tile([128, T], F32, tag="pvproj", bufs=2)
                for th in range(2):
                    sl = slice(512 * th, 512 * (th + 1))
                    nc.tensor.matmul(po[:, sl], wo[:, 2, mt, :],
                                     ctx[:, 2, sl], start=True, stop=True)
                nc.vector.tensor_tensor(y_acc[:, mt, :], po, y_acc[:, mt, :],
                                        op=ALU.add)

        def emit_wo_round_c():
            for mt in range(8):
                po = pa.tile([128, T], F32, tag="pvproj", bufs=2)
                for th in range(2):
                    sl = slice(512 * th, 512 * (th + 1))
                    nc.tensor.matmul(po[:, sl], wo[:, 3, mt, :],
                                     ctx[:, 3, sl], start=True, stop=True)
                ysb = tr.tile([128, T], F32, tag="ysb", bufs=2)
                nc.vector.tensor_tensor(ysb, po, y_acc[:, mt, :], op=ALU.add)
                nc.sync.dma_start(
                    out=YT.ap()[128 * mt:128 * (mt + 1), :], in_=ysb
                )

        # ---------- main emission ----------
        wbuf_of = {}
        exp_insts_of = {g: [] for g in range(4)}
        silu_of = {}
        pending = []  # spliceable scores+exp units of the previous wave

        def drain(n):
            for _ in range(min(n, len(pending))):
                pending.pop(0)()

        # V-projection units interleaved into iterations 1-2 so the PV
        # chains of early waves can run during the ACT stream
        v_sched = {1: [0, 1, 2, 3], 2: [4, 5, 6, 7]}
        for g in range(4):
            # K and Q projections back-to-back: 32 dependency-free matmuls so
            # the PE ramps to full clock; the small dependent matmuls (swap /
            # key_self / m-broadcast) come later when their inputs are ready
            ps_k = proj_chain(wk, g)
            ps_q = proj_chain(wq, g)
            ksb = bias_copy(ps_k, bkt_sb, g)
            qsb_q = bias_copy(ps_q, bqt_sb, g)
            drain(3)
            finish_k_phase(g, ksb)
            drain(3)
            if g - 1 >= 0:
                emit_ln_silu(g - 1)
            drain(6)
            rope_from(qsb_q, g, qrope)
            for tt in v_sched.get(g, []):
                emit_v_unit(tt)
            # this wave's scores: first 4 units now, rest after next K phase
            wbuf_of[g] = pw.tile([128, 2, W_COLS], F16, tag="wbuf",
                                 name=f"wbuf{g}")
            units = score_tile_units(g)
            for fn in units[:4]:
                fn()
            pending = units[4:]
            # early waves' threshold/PV/normalize run while ACT streams on
            if g == 2:
                emit_head_tail(0, 0)
                emit_head_tail(0, 1)
            elif g == 3:
                emit_head_tail(1, 0)
                emit_head_tail(1, 1)

        while pending:
            pending.pop(0)()
        emit_ln_silu(3)

        if dbg:
            nc.sync.dma_start(out=DKR.ap(), in_=krope)
            nc.sync.dma_start(out=DQR.ap(), in_=qrope)
            nc.sync.dma_start(out=DWB.ap(), in_=wbuf_of[0])
        # remaining tails + W_O rounds (warm streams hold the clock through
        # the serial threshold/normalize chains)
        emit_wo_round_a()
        warm_pe(6)
        emit_head_tail(2, 0)
        warm_pe(6)
        emit_head_tail(2, 1)
        warm_pe(6)
        emit_wo_round_b()
        warm_pe(6)
        emit_head_tail(3, 0)
        warm_pe(6)
        emit_head_tail(3, 1)
        warm_pe(6)
        emit_wo_round_c()
        if dbg:
            nc.sync.dma_start(out=DCTX.ap(), in_=ctx)

        pa.release()
        pw.release()
        tr.release()
        pk.release()
        pc.release()

    # Route exp and ln to the combined natural_log_exp_and_others ACT table
    # set (saves one table load + drain per wave): strip those functions from
    # the earlier-indexed single-function sets so the set picker can't choose
    # them. Indices (= act_func_set_id walrus remaps by) stay intact.
    import concourse.bacc as _bacc_mod
    from concourse.hw_specs import get_activation_tables as _gat

    def _gat_patched(arch):
        t = {k: set(v) for k, v in _gat(arch).items()}
        if "natural_log_exp_and_others" in t:
            for k in t:
                if k != "natural_log_exp_and_others":
                    t[k].discard(AF.Exp)
                    t[k].discard(AF.Ln)
        return t

    _bacc_mod.get_activation_tables = _gat_patched
    try:
        nc.finalize()
    finally:
        _bacc_mod.get_activation_tables = _gat
    return nc


def _host_inputs(inputs):
    """Build the 8 per-core input maps from full inputs."""
    X = np.asarray(inputs["X"], dtype=np.float32)
    W_Q = np.asarray(inputs["W_Q"], dtype=np.float32)
    b_Q = np.asarray(inputs["b_Q"], dtype=np.float32)
    W_K = np.asarray(inputs["W_K"], dtype=np.float32)
    b_K = np.asarray(inputs["b_K"], dtype=np.float32)
    W_V = np.asarray(inputs["W_V"], dtype=np.float32)
    b_V = np.asarray(inputs["b_V"], dtype=np.float32)
    sink = np.asarray(inputs["sink_scalars"], dtype=np.float32)
    v_nulls = np.asarray(inputs["v_nulls"], dtype=np.float32)
    W_O = np.asarray(inputs["W_O"], dtype=np.float32)

    XT = np.ascontiguousarray(X[0].T)  # [C, T]

    # channel permutation (evens then odds) within each head's 64 channels
    perm64 = np.concatenate([np.arange(0, 64, 2), np.arange(1, 64, 2)])
    perm512 = (np.arange(8)[:, None] * 64 + perm64[None, :]).reshape(-1)

    # RoPE tables, matching reference float32 math
    invf = (1.0 / (10000.0 ** (np.arange(0, DH, 2, dtype=np.float32) / DH))).astype(
        np.float32
    )
    freqs = np.arange(T, dtype=np.float32)[:, None] * invf[None, :]  # [T, 32]
    cos32 = np.cos(freqs).T  # [32, T]
    sin32 = np.sin(freqs).T
    cos128 = np.tile(cos32, (4, 1)).astype(np.float16)
    sin128 = np.concatenate([-sin32, sin32, -sin32, sin32], axis=0).astype(np.float16)

    # swap matrix: out[p] = q[partner(p)]; lhsT[p', p] = 1 iff p' = partner(p)
    pswap = np.zeros((128, 128), dtype=np.float16)
    for p in range(128):
        partner = p + 32 if (p % 64) < 32 else p - 32
        pswap[partner, p] = 1.0

    # f16 constant blob
    cb16 = np.zeros((128, CB16_COLS), dtype=np.float16)
    cb16[0, CB_ONES:CB_ONES + 512] = 1.0
    cb16[:, CB_PSW:CB_PSW + 128] = pswap
    for g in range(4):
        cb16[0:64, CB_SEL2 + 2 * g] = 1.0
        cb16[64:128, CB_SEL2 + 2 * g + 1] = 1.0
    cb16[0, CB_BCH:CB_BCH + 64] = 1.0
    cb16[1, CB_BCH + 64:CB_BCH + 128] = 1.0
    cb16[:, CB_COS:CB_COS + T] = cos128
    cb16[:, CB_SIN:CB_SIN + T] = sin128
    cb16[:, CB_IDF:CB_IDF + 128] = np.eye(128, dtype=np.float16)
    mask = np.zeros((128, 128), dtype=np.float16)
    for c in range(128):
        mask[c, :c] = -60000.0  # key c masks queries j < c
    cb16[:, CB_MSK:CB_MSK + 128] = mask

    in_maps = []
    for c in range(N_CORES):
        n, half = c // 2, c % 2
        qs = slice(512 * c, 512 * (c + 1))
        ks = slice(512 * half, 512 * (half + 1))
        heads = np.arange(8 * c, 8 * c + 8)
        sinks = sink[heads]  # [8]
        cb16_c = cb16.copy()
        cb16_c[0, CB_BV:CB_BV + 512] = b_V[ks].astype(np.float16)
        vn = v_nulls[n].reshape(N_HEAD, DH)  # base-head x d
        cb32 = np.zeros((128, 32), dtype=np.float32)
        cb32[:, 0:8] = np.tile((S * sinks)[None, :], (128, 1))
        cb32[0, 8:16] = S * (sinks + 1e-6)
        for h in range(8):
            bh = (8 * half) + h  # base head index within branch
            cb32[0:64, 16 + h] = S * sinks[h] * vn[bh]
        cb32[:, 24:28] = b_Q[qs][perm512].reshape(4, 128).T
        cb32[:, 28:32] = b_K[ks][perm512].reshape(4, 128).T
        in_maps.append(
            {
                "XT": XT.astype(np.float16),
                "WQ": np.ascontiguousarray(W_Q[:, qs][:, perm512]).astype(np.float16),
                "WK": np.ascontiguousarray(W_K[:, ks][:, perm512]).astype(np.float16),
                "WV": np.ascontiguousarray(W_V[:, ks]).astype(np.float16),
                "CB16": cb16_c,
                "CB32": cb32,
                "WO": np.ascontiguousarray(0.25 * W_O[n, ks, :]).astype(np.float16),
            }
        )
    return in_maps


def kernel(**inputs) -> np.ndarray:
    from concourse.bass_utils import run_bass_kernel_spmd

    in_maps = _host_inputs(inputs)
    if _NC_CACHE[0] is None:
        _NC_CACHE[0] = _build_nc()
    nc = _NC_CACHE[0]
    trace = bool(os.environ.get("KBENCH_TRACE"))
    res = run_bass_kernel_spmd(
        nc, in_maps, core_ids=list(range(N_CORES)), trace=trace
    )
    LAST_RESULT[0] = res
    if trace and res.exec_time_ns is not None:
        print(f"HW exec time: {res.exec_time_ns} ns")

    W_O_bias = np.asarray(inputs["W_O_bias"], dtype=np.float32)
    y = np.zeros((T, D_MODEL), dtype=np.float32)
    for r in res.results:
        y += r["YT"].T
    y += W_O_bias.mean(axis=0)[None, :]
    return y[None, :, :]
